# revision 1
# baseline (speedup 1.0000x reference)
"""Locally-connected 2D layer on 8 Trainium2 NeuronCores.

Problem: x[128,3,64,64] f32, per-position weights W[60,60,32,75], bias b[60,60,32]
  out[b,o,y,x] = sum_k patches[b,y,x,k] * W[y,x,o,k] + b[y,x,o],  k=(c,dy,dx)

Strategy (spatial sharding over output rows, 8 rows/core, memory-regime):
  - The contraction (c,dy,dx)=75 must live on SBUF partitions for the PE. dy is
    handled with a mod-5 ring of "patch planes" XP[(r%5, c, dx), x*128+b]; the
    per-row dy rotation is folded into the HOST-side W layout (np.roll), so the
    device always reads XP[0:76] as one contiguous partition range.
  - Ring planes are pre-replicated on the HOST (dx-im2col) into xpr[12,15,FXB]
    so every device fill is a plain [15, 30KB] DRAM->SBUF slice copy; fills are
    split into 4 free-chunks gated on the matmul chunks that last read the
    slot, so the ring advance overlaps the row's own compute.
  - Bias is folded in as contraction row 75 (W row 75 = bias, XP row 75 = 1.0).
  - Per output row: 15 groups of 4 column-tiled matmuls (lhsT=W[76,32],
    rhs=XP[76,128] -> out[32o,128b] at PSUM partitions 32j), PSUM->SBUF via DVE,
    one 983KB store per row in a DMA-friendly layout; host re-transposes once.
"""

import numpy as np

B, C, H, WIDTH = 128, 3, 64, 64
KH = KW = 5
RY = RX = 60
O = 32
K = 75
NCORES = 8
RPC = 8             # output rows computed per core (8*8=64, last 4 dropped)
INR = RPC + KH - 1  # 12 input rows per core
PADH = NCORES * RPC + KH - 1  # 68
NG = 15             # groups of 4 x-positions per row
CHUNKS = ((0, 4), (4, 4), (8, 4), (12, 3))  # (first group, n groups) per PSUM chunk
FXB = RX * B        # 7680 elements per patch plane

_cache = {}

USE_BF16 = True  # inputs (x-planes, W) in bf16; accumulation + output stay f32


def _build():
    import concourse.bass as bass
    import concourse.bacc as bacc
    import concourse.tile as tile
    import concourse.mybir as mybir

    f32 = mybir.dt.float32
    din = mybir.dt.bfloat16 if USE_BF16 else f32
    nc = bacc.Bacc("TRN2", target_bir_lowering=False, debug=False,
                   num_devices=NCORES)
    xpr_d = nc.dram_tensor("xpr", [INR, KH * C, FXB], din, kind="ExternalInput")
    wh_d = nc.dram_tensor("wh", [RPC, K + 1, RX, O], din, kind="ExternalInput")
    ones_d = nc.dram_tensor("ones", [1, FXB], din, kind="ExternalInput")
    oc_d = nc.dram_tensor("oc", [RPC, 4, O, NG, B], f32, kind="ExternalOutput")

    NPL = KH * C  # 15 planes per input row

    with tile.TileContext(nc) as tc:
        with (
            tc.tile_pool(name="const", bufs=1) as cpool,
            tc.tile_pool(name="w", bufs=4) as wpool,
            tc.tile_pool(name="os", bufs=2) as opool,
            tc.tile_pool(name="ps", bufs=4, space=bass.MemorySpace.PSUM) as ppool,
        ):
            xp = cpool.tile([K + 1, FXB], din)  # [76, 7680]; row 75 = ones

            nc.sync.dma_start(xp[K:K + 1, :], ones_d[:])
            for r in range(KH):  # initial ring: rows 0..4 -> slots 0..4
                nc.gpsimd.dma_start(xp[r * NPL:(r + 1) * NPL, :], xpr_d[r])

            wts = {}

            def load_w(k):
                wts[k] = wpool.tile([K + 1, RX * O], din, name="wt", tag="wt")
                nc.gpsimd.dma_start(wts[k][:],
                                    wh_d[k].rearrange("k x o -> k (x o)"))

            load_w(0)
            load_w(1)

            for k in range(RPC):
                wt = wts.pop(k)
                ot = opool.tile([128, NG * B], f32)  # [128, 1920]
                for ci, (g0, gn) in enumerate(CHUNKS):
                    pt = ppool.tile([128, 4 * B], f32)
                    for gs in range(gn):
                        for j in range(4):
                            xpos = (g0 + gs) * 4 + j
                            nc.tensor.matmul(
                                pt[32 * j:32 * (j + 1), gs * B:(gs + 1) * B],
                                wt[:, xpos * O:(xpos + 1) * O],
                                xp[:, xpos * B:(xpos + 1) * B],
                                tile_position=(0, 32 * j),
                            )
                    nc.vector.tensor_copy(
                        ot[:, g0 * B:(g0 + gn) * B], pt[:, :gn * B])
                    if k + KH < INR:
                        # ring advance for row k+1: overwrite slot k%5 with
                        # input row k+5, chunk-gated on this chunk's matmuls
                        slot = k % KH
                        f0, f1 = g0 * 4 * B, (g0 + gn) * 4 * B
                        nc.gpsimd.dma_start(
                            xp[slot * NPL:(slot + 1) * NPL, f0:f1],
                            xpr_d[k + KH, :, f0:f1])
                if k + 2 < RPC:
                    load_w(k + 2)
                nc.scalar.dma_start(
                    oc_d[k].rearrange("j o g b -> (j o) (g b)"), ot[:])

    nc.compile()
    return nc


def _get_nc():
    if "nc" not in _cache:
        _cache["nc"] = _build()
    return _cache["nc"]


def _prep_inputs(x, W, b):
    x = np.asarray(x, np.float32)
    W = np.asarray(W, np.float32)
    b = np.asarray(b, np.float32)
    xh = np.zeros((PADH, C, WIDTH, B), np.float32)
    xh[:H] = x.transpose(2, 1, 3, 0)  # [row, c, w, batch]
    # ring planes: xpr_full[r, (c,dx) -> c*KW+dx, x, b] = xh[r, c, x+dx, b]
    # plane order within a slot must be p2 = c*KW + dx (with slot-major rm)
    xpr_full = np.zeros((PADH, C, KW, RX, B), np.float32)
    for dx in range(KW):
        xpr_full[:, :, dx] = xh[:, :, dx:dx + RX]
    xpr_full = xpr_full.reshape(PADH, C * KW, FXB)
    Wfull = W.transpose(0, 3, 1, 2)  # [RY, K, RX, O]
    in_maps = []
    for i in range(NCORES):
        whc = np.zeros((RPC, K + 1, RX, O), np.float32)
        for k in range(RPC):
            y = RPC * i + k
            if y < RY:
                w5 = Wfull[y].reshape(C, KH, KW, RX, O)
                # device slot rm holds input row with (local row)%5 == rm;
                # slot rm supplies dy=(rm-k)%5 for output row k -> roll by k.
                # partition order: p = rm*15 + c*5 + dx
                whc[k, :K] = np.roll(w5, k, axis=1).transpose(1, 0, 2, 3, 4) \
                    .reshape(K, RX, O)
                whc[k, K] = b[y]
        if USE_BF16:
            import ml_dtypes
            bf = ml_dtypes.bfloat16
            in_maps.append({
                "xpr": np.ascontiguousarray(
                    xpr_full[RPC * i:RPC * i + INR]).astype(bf),
                "wh": whc.astype(bf),
                "ones": np.ones((1, FXB), bf),
            })
        else:
            in_maps.append({
                "xpr": np.ascontiguousarray(xpr_full[RPC * i:RPC * i + INR]),
                "wh": whc,
                "ones": np.ones((1, FXB), np.float32),
            })
    return in_maps


def kernel(x, W, b):
    from concourse.bass_utils import run_bass_kernel_spmd

    nc = _get_nc()
    in_maps = _prep_inputs(x, W, b)
    br = run_bass_kernel_spmd(nc, in_maps, list(range(NCORES)),
                              **_cache.get("run_kwargs", {}))
    _cache["last_run"] = br
    oc = np.stack([np.asarray(br.results[i]["oc"]) for i in range(NCORES)])
    oc = oc.reshape(NCORES * RPC, 4, O, NG, B)  # [64, j, o, x4, b]
    out = oc.transpose(4, 2, 0, 3, 1).reshape(B, O, NCORES * RPC, RX)
    return np.ascontiguousarray(out[:, :, :RY, :])



# revision 5
# speedup vs baseline: 1.2763x; 1.2763x over previous
"""Locally-connected 2D layer on 8 Trainium2 NeuronCores.

Problem: x[128,3,64,64] f32, per-position weights W[60,60,32,75], bias b[60,60,32]
  out[b,o,y,x] = sum_k patches[b,y,x,k] * W[y,x,o,k] + b[y,x,o],  k=(c,dy,dx)

Strategy (spatial sharding over output rows, 8 rows/core, memory-regime):
  - Groups of 4 consecutive x positions share one full-array matmul: the
    contraction is the UNION of the 4 patch windows, planes (c, dy, dx') with
    dx' in 0..7 -> 5*3*8 = 120 partitions (+1 bias row = 121).  The stationary
    [121, 128] holds all 4 positions' weights ((j,o) columns, structural zeros
    where dx'-j is outside 0..4), so each [128, 128] matmul output is fully
    useful: out[(j,o), b] for 4 x-positions at once.  15 matmuls per row,
    120 per core, N=128 streaming, FWL-eligible (128-col bf16 stationary).
  - dy is handled with a mod-5 ring of row-slots (24 planes each); the per-row
    dy rotation is folded into the host-side W slab layout (np.roll), so the
    device always reads xp[0:121] as one fixed partition range.
  - DMA layout tuned for few, wide transfers: W slab in 4 paired-row DMAs of
    928KB on the sync HWDGE queue; initial 5-row ring fill is one 460KB DMA on
    scalar; ring advances are [24, 3840B] rows on gpsimd (SWDGE), split in two
    chunk-gated halves; output staged in SBUF as bf16 and stored per row
    (492KB, [128, 3840B] lines) on scalar.  PSUM evacuation alternates
    vector/scalar engines; output is bf16 (host upcasts to f32).
"""

import numpy as np

B, C, H, WIDTH = 128, 3, 64, 64
KH = KW = 5
RY = RX = 60
O = 32
K = 75
NCORES = 8
RPC = 8             # output rows computed per core (8*8=64, last 4 dropped)
INR = RPC + KH - 1  # 12 input rows per core
PADH = NCORES * RPC + KH - 1  # 68
NG = 15             # groups of 4 x-positions per row
NPL = C * 8         # 24 planes per ring slot (c, dx' in 0..7)
KP = KH * NPL + 1   # 121 contraction partitions (120 planes + bias)
FU = NG * B         # 1920 free elems per plane (g, b)
CHUNKS = ((0, 4), (4, 4), (8, 4), (12, 3))  # (first group, n groups) per PSUM chunk

_cache = {}


def _build():
    import concourse.bass as bass
    import concourse.bacc as bacc
    import concourse.tile as tile
    import concourse.mybir as mybir

    f32 = mybir.dt.float32
    din = mybir.dt.bfloat16
    nc = bacc.Bacc("TRN2", target_bir_lowering=False, debug=False,
                   num_devices=NCORES)
    u_d = nc.dram_tensor("u", [INR, NPL, FU], din, kind="ExternalInput")
    w_d = nc.dram_tensor("w", [KP, RPC * NG * B], din, kind="ExternalInput")
    oc_d = nc.dram_tensor("oc", [RPC, 4, O, NG, B], din, kind="ExternalOutput")

    with tile.TileContext(nc) as tc:
        with (
            tc.tile_pool(name="const", bufs=1) as cpool,
            tc.tile_pool(name="os", bufs=3) as opool,
            tc.tile_pool(name="ps", bufs=4, space=bass.MemorySpace.PSUM) as ppool,
        ):
            xp = cpool.tile([KP, FU], din)  # ring planes + ones row 120
            ws = cpool.tile([KP, RPC * NG * B], din)

            # ones row lives at partition 120; engine ops need a 32-aligned
            # partition base, so memset 96-120 and let the ring fill overwrite
            # partitions 96-119 with real plane data afterwards
            nc.vector.memset(xp[96:KP, :], 1.0)
            nc.scalar.dma_start(xp[0:KH * NPL, :],
                                u_d[0:KH].rearrange("r p f -> (r p) f"))
            for p in range(4):  # W row-pairs on the sync HWDGE queue
                c0, c1 = p * 2 * NG * B, (p + 1) * 2 * NG * B
                nc.sync.dma_start(ws[:, c0:c1], w_d[:, c0:c1])

            for k in range(RPC):
                ot = opool.tile([128, FU], din)
                for ci, (g0, gn) in enumerate(CHUNKS):
                    pt = ppool.tile([128, 4 * B], f32)
                    for gg in range(gn):
                        g = g0 + gg
                        nc.tensor.matmul(
                            pt[:, gg * B:(gg + 1) * B],
                            ws[:, (k * NG + g) * B:(k * NG + g + 1) * B],
                            xp[:, g * B:(g + 1) * B],
                        )
                    if ci % 2 == 0:
                        nc.vector.tensor_copy(
                            ot[:, g0 * B:(g0 + gn) * B], pt[:, :gn * B])
                    else:
                        nc.scalar.copy(
                            ot[:, g0 * B:(g0 + gn) * B], pt[:, :gn * B])
                    if k + KH < INR and ci in (1, 3):
                        # ring advance for rows k+1..: overwrite slot k%5 with
                        # input row k+5 in two chunk-gated halves
                        slot = k % KH
                        f0, f1 = (0, 8 * B) if ci == 1 else (8 * B, FU)
                        nc.gpsimd.dma_start(
                            xp[slot * NPL:(slot + 1) * NPL, f0:f1],
                            u_d[k + KH, :, f0:f1])
                nc.scalar.dma_start(
                    oc_d[k].rearrange("j o g b -> (j o) (g b)"), ot[:])

    nc.compile()
    return nc


def _get_nc():
    if "nc" not in _cache:
        _cache["nc"] = _build()
    return _cache["nc"]


def _prep_inputs(x, W, b):
    import ml_dtypes
    bf = ml_dtypes.bfloat16
    x = np.asarray(x, np.float32)
    W = np.asarray(W, np.float32)
    b = np.asarray(b, np.float32)
    xh = np.zeros((PADH, C, WIDTH, B), np.float32)
    xh[:H] = x.transpose(2, 1, 3, 0)  # [row, c, w, batch]
    # union planes: U[row, (c,dx'), (g,b)] = xh[row, c, 4g+dx', b], dx' in 0..7
    U = np.zeros((PADH, C, 8, NG, B), np.float32)
    for dxp in range(8):
        U[:, :, dxp] = xh[:, :, dxp::4][:, :, :NG]
    U = U.reshape(PADH, NPL, FU).astype(bf)

    W5 = W.reshape(RY, RX, O, C, KH, KW)
    in_maps = []
    for i in range(NCORES):
        nk = min(RPC, RY - RPC * i)
        W5c = np.zeros((RPC, NG, 4, O, C, KH, KW), np.float32)
        bc = np.zeros((RPC, NG, 4, O), np.float32)
        W5c[:nk] = W5[RPC * i:RPC * i + nk].reshape(nk, NG, 4, O, C, KH, KW)
        bc[:nk] = b[RPC * i:RPC * i + nk].reshape(nk, NG, 4, O)
        A = W5c.transpose(5, 4, 6, 0, 1, 2, 3)  # [dy, c, dx, k, g, j, o]
        D = np.zeros((KH, C, 8, RPC, NG, 4, O), np.float32)
        for j in range(4):
            D[:, :, j:j + KW, :, :, j, :] = A[:, :, :, :, :, j, :]
        S = np.empty_like(D)  # slot rm holds dy=(rm-k)%5 -> roll dy by k
        for k in range(RPC):
            S[:, :, :, k] = np.roll(D[:, :, :, k], k, axis=0)
        wslab = np.zeros((KP, RPC * NG * 4 * O), np.float32)
        wslab[:KH * NPL] = S.reshape(KH * NPL, -1)
        wslab[KH * NPL] = bc.reshape(-1)
        in_maps.append({
            "u": np.ascontiguousarray(U[RPC * i:RPC * i + INR]),
            "w": wslab.astype(bf),
        })
    return in_maps


def kernel(x, W, b):
    from concourse.bass_utils import run_bass_kernel_spmd

    nc = _get_nc()
    in_maps = _prep_inputs(x, W, b)
    br = run_bass_kernel_spmd(nc, in_maps, list(range(NCORES)),
                              **_cache.get("run_kwargs", {}))
    _cache["last_run"] = br
    oc = np.stack([np.asarray(br.results[i]["oc"]) for i in range(NCORES)])
    oc = oc.reshape(NCORES * RPC, 4, O, NG, B).astype(np.float32)
    out = oc.transpose(4, 2, 0, 3, 1).reshape(B, O, NCORES * RPC, RX)
    return np.ascontiguousarray(out[:, :, :RY, :])


# revision 6
# speedup vs baseline: 1.4078x; 1.1030x over previous
"""Locally-connected 2D layer on 8 Trainium2 NeuronCores.

Problem: x[128,3,64,64] f32, per-position weights W[60,60,32,75], bias b[60,60,32]
  out[b,o,y,x] = sum_k patches[b,y,x,k] * W[y,x,o,k] + b[y,x,o],  k=(c,dy,dx)

Strategy (spatial sharding over output rows, 8 rows/core, memory-regime):
  - Groups of 4 consecutive x positions share one full-array matmul: the
    contraction is the UNION of the 4 patch windows, planes (c, dy, dx') with
    dx' in 0..7 -> 5*3*8 = 120 partitions (+1 bias row = 121).  The stationary
    [121, 128] holds all 4 positions' weights ((j,o) columns, structural zeros
    where dx'-j is outside 0..4), so each [128, 128] matmul output is fully
    useful: out[(j,o), b] for 4 x-positions at once.  15 matmuls per row,
    120 per core, N=128 streaming, FWL-eligible (128-col bf16 stationary).
  - dy is handled with a mod-5 ring of row-slots (24 planes each); the per-row
    dy rotation is folded into the host-side W slab layout (np.roll), so the
    device always reads xp[0:121] as one fixed partition range.
  - All input HBM traffic is a few wide DMAs: initial ring fill [120, 3840B]
    and future-row slab [120, 7680B] on the sync HWDGE queue, W slab (padded
    to 128 partitions) in four [128, 7680B] row-pair DMAs on the gpsimd SWDGE
    queue.  Ring advances are SBUF->SBUF copies from the staged future slab
    (no mid-kernel HBM input), chunk-gated in two halves.
  - Output is bf16 (host upcasts): each row's [128, 3840B] store is split in
    two chunk-gated halves alternating between the sync and scalar queues.
    PSUM evacuation alternates vector/scalar engines.
"""

import numpy as np

B, C, H, WIDTH = 128, 3, 64, 64
KH = KW = 5
RY = RX = 60
O = 32
K = 75
NCORES = 8
RPC = 8             # output rows computed per core (8*8=64, last 4 dropped)
INR = RPC + KH - 1  # 12 input rows per core
PADH = NCORES * RPC + KH - 1  # 68
NG = 15             # groups of 4 x-positions per row
NPL = C * 8         # 24 planes per ring slot (c, dx' in 0..7)
KP = KH * NPL + 1   # 121 contraction partitions (120 planes + bias)
FU = NG * B         # 1920 free elems per plane (g, b)
CHUNKS = ((0, 4), (4, 4), (8, 4), (12, 3))  # (first group, n groups) per PSUM chunk

_cache = {}


def _build():
    import concourse.bass as bass
    import concourse.bacc as bacc
    import concourse.tile as tile
    import concourse.mybir as mybir

    f32 = mybir.dt.float32
    din = mybir.dt.bfloat16
    nc = bacc.Bacc("TRN2", target_bir_lowering=False, debug=False,
                   num_devices=NCORES)
    ui_d = nc.dram_tensor("ui", [KH * NPL, FU], din, kind="ExternalInput")
    uf_d = nc.dram_tensor("uf", [KH * NPL, 2 * FU], din, kind="ExternalInput")
    w_d = nc.dram_tensor("w", [128, RPC * NG * B], din, kind="ExternalInput")
    oc_d = nc.dram_tensor("oc", [RPC, 4, O, NG, B], din, kind="ExternalOutput")

    with tile.TileContext(nc) as tc:
        with (
            tc.tile_pool(name="const", bufs=1) as cpool,
            tc.tile_pool(name="os", bufs=3) as opool,
            tc.tile_pool(name="ps", bufs=4, space=bass.MemorySpace.PSUM) as ppool,
        ):
            xp = cpool.tile([KP, FU], din)       # ring planes + ones row 120
            uf = cpool.tile([KH * NPL, 2 * FU], din)  # future rows 5-11 staged
            ws = cpool.tile([128, RPC * NG * B], din)

            # ones row lives at partition 120; engine ops need a 32-aligned
            # partition base, so memset 96-120 and let the ring fill overwrite
            # partitions 96-119 with real plane data afterwards
            nc.vector.memset(xp[96:KP, :], 1.0)
            nc.sync.dma_start(xp[0:KH * NPL, :], ui_d[:])
            nc.sync.dma_start(uf[:], uf_d[:])
            for p in range(4):  # W row-pairs on the gpsimd SWDGE queue
                c0, c1 = p * 2 * NG * B, (p + 1) * 2 * NG * B
                nc.gpsimd.dma_start(ws[:, c0:c1], w_d[:, c0:c1])

            for k in range(RPC):
                ot = opool.tile([128, FU], din)
                oc_k = oc_d[k].rearrange("j o g b -> (j o) (g b)")
                st_eng = nc.sync if k % 2 == 0 else nc.scalar
                for ci, (g0, gn) in enumerate(CHUNKS):
                    pt = ppool.tile([128, 4 * B], f32)
                    for gg in range(gn):
                        g = g0 + gg
                        nc.tensor.matmul(
                            pt[:, gg * B:(gg + 1) * B],
                            ws[0:KP, (k * NG + g) * B:(k * NG + g + 1) * B],
                            xp[:, g * B:(g + 1) * B],
                        )
                    if ci % 2 == 0:
                        nc.vector.tensor_copy(
                            ot[:, g0 * B:(g0 + gn) * B], pt[:, :gn * B])
                    else:
                        nc.scalar.copy(
                            ot[:, g0 * B:(g0 + gn) * B], pt[:, :gn * B])
                        f0, f1 = (0, 8 * B) if ci == 1 else (8 * B, FU)
                        if k + KH < INR:
                            # ring advance: slot k%5 <- input row k+5, copied
                            # SBUF->SBUF from the staged future slab
                            s = k % KH
                            go = 0 if k + KH < 2 * KH else FU
                            nc.gpsimd.dma_start(
                                xp[s * NPL:(s + 1) * NPL, f0:f1],
                                uf[s * NPL:(s + 1) * NPL, go + f0:go + f1])
                        st_eng.dma_start(oc_k[:, f0:f1], ot[:, f0:f1])

    nc.compile()
    return nc


def _get_nc():
    if "nc" not in _cache:
        _cache["nc"] = _build()
    return _cache["nc"]


def _prep_inputs(x, W, b):
    import ml_dtypes
    bf = ml_dtypes.bfloat16
    x = np.asarray(x, np.float32)
    W = np.asarray(W, np.float32)
    b = np.asarray(b, np.float32)
    xh = np.zeros((PADH, C, WIDTH, B), np.float32)
    xh[:H] = x.transpose(2, 1, 3, 0)  # [row, c, w, batch]
    # union planes: U[row, (c,dx'), (g,b)] = xh[row, c, 4g+dx', b], dx' in 0..7
    U = np.zeros((PADH, C, 8, NG, B), np.float32)
    for dxp in range(8):
        U[:, :, dxp] = xh[:, :, dxp::4][:, :, :NG]
    U = U.reshape(PADH, NPL, FU).astype(bf)

    W5 = W.reshape(RY, RX, O, C, KH, KW)
    in_maps = []
    for i in range(NCORES):
        nk = min(RPC, RY - RPC * i)
        W5c = np.zeros((RPC, NG, 4, O, C, KH, KW), np.float32)
        bc = np.zeros((RPC, NG, 4, O), np.float32)
        W5c[:nk] = W5[RPC * i:RPC * i + nk].reshape(nk, NG, 4, O, C, KH, KW)
        bc[:nk] = b[RPC * i:RPC * i + nk].reshape(nk, NG, 4, O)
        A = W5c.transpose(5, 4, 6, 0, 1, 2, 3)  # [dy, c, dx, k, g, j, o]
        D = np.zeros((KH, C, 8, RPC, NG, 4, O), np.float32)
        for j in range(4):
            D[:, :, j:j + KW, :, :, j, :] = A[:, :, :, :, :, j, :]
        S = np.empty_like(D)  # slot rm holds dy=(rm-k)%5 -> roll dy by k
        for k in range(RPC):
            S[:, :, :, k] = np.roll(D[:, :, :, k], k, axis=0)
        wslab = np.zeros((128, RPC * NG * 4 * O), np.float32)
        wslab[:KH * NPL] = S.reshape(KH * NPL, -1)
        wslab[KH * NPL] = bc.reshape(-1)

        Uc = U[RPC * i:RPC * i + INR]  # [12, 24, FU]
        ufut = np.zeros((KH * NPL, 2 * FU), bf)
        for r in range(KH, INR):
            s, go = r % KH, (0 if r < 2 * KH else 1)
            ufut[s * NPL:(s + 1) * NPL, go * FU:(go + 1) * FU] = Uc[r]
        in_maps.append({
            "ui": np.ascontiguousarray(Uc[:KH].reshape(KH * NPL, FU)),
            "uf": ufut,
            "w": wslab.astype(bf),
        })
    return in_maps


def kernel(x, W, b):
    from concourse.bass_utils import run_bass_kernel_spmd

    nc = _get_nc()
    in_maps = _prep_inputs(x, W, b)
    br = run_bass_kernel_spmd(nc, in_maps, list(range(NCORES)),
                              **_cache.get("run_kwargs", {}))
    _cache["last_run"] = br
    oc = np.stack([np.asarray(br.results[i]["oc"]) for i in range(NCORES)])
    oc = oc.reshape(NCORES * RPC, 4, O, NG, B).astype(np.float32)
    out = oc.transpose(4, 2, 0, 3, 1).reshape(B, O, NCORES * RPC, RX)
    return np.ascontiguousarray(out[:, :, :RY, :])


# revision 7
# speedup vs baseline: 1.4104x; 1.0018x over previous
"""Locally-connected 2D layer on 8 Trainium2 NeuronCores.

Problem: x[128,3,64,64] f32, per-position weights W[60,60,32,75], bias b[60,60,32]
  out[b,o,y,x] = sum_k patches[b,y,x,k] * W[y,x,o,k] + b[y,x,o],  k=(c,dy,dx)

Strategy (spatial sharding over output rows, 8 rows/core, memory-regime):
  - Groups of 4 consecutive x positions share one full-array matmul: the
    contraction is the UNION of the 4 patch windows, planes (c, dy, dx') with
    dx' in 0..7 -> 5*3*8 = 120 partitions (+1 bias row = 121).  The stationary
    [121, 128] holds all 4 positions' weights ((j,o) columns, structural zeros
    where dx'-j is outside 0..4), so each [128, 128] matmul output is fully
    useful: out[(j,o), b] for 4 x-positions at once.  15 matmuls per row,
    120 per core, N=128 streaming, FWL-eligible (128-col bf16 stationary).
  - dy is handled with a mod-5 ring of row-slots (24 planes each); the per-row
    dy rotation is folded into the host-side W slab layout (np.roll), so the
    device always reads xp[0:121] as one fixed partition range.
  - All input HBM traffic is a few wide DMAs: initial ring fill [120, 3840B]
    and future-row slab [120, 7680B] on the sync HWDGE queue, W slab (padded
    to 128 partitions) in four [128, 7680B] row-pair DMAs on the gpsimd SWDGE
    queue.  Ring advances are SBUF->SBUF copies from the staged future slab
    (no mid-kernel HBM input), chunk-gated in two halves.
  - Output is bf16 (host upcasts): each row's [128, 3840B] store is split in
    two chunk-gated halves alternating between the sync and scalar queues.
    PSUM evacuation alternates vector/scalar engines.
"""

import numpy as np

B, C, H, WIDTH = 128, 3, 64, 64
KH = KW = 5
RY = RX = 60
O = 32
K = 75
NCORES = 8
RPC = 8             # output rows computed per core (8*8=64, last 4 dropped)
INR = RPC + KH - 1  # 12 input rows per core
PADH = NCORES * RPC + KH - 1  # 68
NG = 15             # groups of 4 x-positions per row
NPL = C * 8         # 24 planes per ring slot (c, dx' in 0..7)
KP = KH * NPL + 1   # 121 contraction partitions (120 planes + bias)
FU = NG * B         # 1920 free elems per plane (g, b)
CHUNKS = ((0, 4), (4, 4), (8, 4), (12, 3))  # (first group, n groups) per PSUM chunk

_cache = {}


def _build():
    import concourse.bass as bass
    import concourse.bacc as bacc
    import concourse.tile as tile
    import concourse.mybir as mybir

    f32 = mybir.dt.float32
    din = mybir.dt.bfloat16
    nc = bacc.Bacc("TRN2", target_bir_lowering=False, debug=False,
                   num_devices=NCORES)
    ui_d = nc.dram_tensor("ui", [KH * NPL, FU], din, kind="ExternalInput")
    uf_d = nc.dram_tensor("uf", [KH * NPL, 2 * FU], din, kind="ExternalInput")
    w_d = nc.dram_tensor("w", [128, RPC * NG * B], din, kind="ExternalInput")
    oc_d = nc.dram_tensor("oc", [RPC, 4, O, NG, B], din, kind="ExternalOutput")

    with tile.TileContext(nc) as tc:
        with (
            tc.tile_pool(name="const", bufs=1) as cpool,
            tc.tile_pool(name="os", bufs=3) as opool,
            tc.tile_pool(name="ps", bufs=4, space=bass.MemorySpace.PSUM) as ppool,
        ):
            xp = cpool.tile([KP, FU], din)       # ring planes + ones row 120
            uf = cpool.tile([KH * NPL, 2 * FU], din)  # future rows 5-11 staged
            ws = cpool.tile([128, RPC * NG * B], din)

            # ones row lives at partition 120; engine ops need a 32-aligned
            # partition base, so memset 96-120 and let the ring fill overwrite
            # partitions 96-119 with real plane data afterwards
            nc.vector.memset(xp[96:KP, :], 1.0)
            nc.sync.dma_start(xp[0:KH * NPL, :], ui_d[:])
            # future-row slab split per generation so ring advances only wait
            # on the generation they read
            nc.scalar.dma_start(uf[:, 0:FU], uf_d[:, 0:FU])
            nc.scalar.dma_start(uf[:, FU:2 * FU], uf_d[:, FU:2 * FU])
            for p in range(4):  # W row-pairs on the gpsimd SWDGE queue
                c0, c1 = p * 2 * NG * B, (p + 1) * 2 * NG * B
                nc.gpsimd.dma_start(ws[:, c0:c1], w_d[:, c0:c1])

            for k in range(RPC):
                ot = opool.tile([128, FU], din)
                oc_k = oc_d[k].rearrange("j o g b -> (j o) (g b)")
                st_eng = nc.sync if k % 2 == 0 else nc.scalar
                for ci, (g0, gn) in enumerate(CHUNKS):
                    pt = ppool.tile([128, 4 * B], f32)
                    for gg in range(gn):
                        g = g0 + gg
                        nc.tensor.matmul(
                            pt[:, gg * B:(gg + 1) * B],
                            ws[0:KP, (k * NG + g) * B:(k * NG + g + 1) * B],
                            xp[:, g * B:(g + 1) * B],
                        )
                    if ci % 2 == 0:
                        nc.vector.tensor_copy(
                            ot[:, g0 * B:(g0 + gn) * B], pt[:, :gn * B])
                    else:
                        nc.scalar.copy(
                            ot[:, g0 * B:(g0 + gn) * B], pt[:, :gn * B])
                        f0, f1 = (0, 8 * B) if ci == 1 else (8 * B, FU)
                        if k + KH < INR:
                            # ring advance: slot k%5 <- input row k+5, copied
                            # SBUF->SBUF from the staged future slab
                            s = k % KH
                            go = 0 if k + KH < 2 * KH else FU
                            nc.gpsimd.dma_start(
                                xp[s * NPL:(s + 1) * NPL, f0:f1],
                                uf[s * NPL:(s + 1) * NPL, go + f0:go + f1])
                        st_eng.dma_start(oc_k[:, f0:f1], ot[:, f0:f1])

    nc.compile()
    return nc


def _get_nc():
    if "nc" not in _cache:
        _cache["nc"] = _build()
    return _cache["nc"]


def _prep_inputs(x, W, b):
    import ml_dtypes
    bf = ml_dtypes.bfloat16
    x = np.asarray(x, np.float32)
    W = np.asarray(W, np.float32)
    b = np.asarray(b, np.float32)
    xh = np.zeros((PADH, C, WIDTH, B), np.float32)
    xh[:H] = x.transpose(2, 1, 3, 0)  # [row, c, w, batch]
    # union planes: U[row, (c,dx'), (g,b)] = xh[row, c, 4g+dx', b], dx' in 0..7
    U = np.zeros((PADH, C, 8, NG, B), np.float32)
    for dxp in range(8):
        U[:, :, dxp] = xh[:, :, dxp::4][:, :, :NG]
    U = U.reshape(PADH, NPL, FU).astype(bf)

    W5 = W.reshape(RY, RX, O, C, KH, KW)
    in_maps = []
    for i in range(NCORES):
        nk = min(RPC, RY - RPC * i)
        W5c = np.zeros((RPC, NG, 4, O, C, KH, KW), np.float32)
        bc = np.zeros((RPC, NG, 4, O), np.float32)
        W5c[:nk] = W5[RPC * i:RPC * i + nk].reshape(nk, NG, 4, O, C, KH, KW)
        bc[:nk] = b[RPC * i:RPC * i + nk].reshape(nk, NG, 4, O)
        A = W5c.transpose(5, 4, 6, 0, 1, 2, 3)  # [dy, c, dx, k, g, j, o]
        D = np.zeros((KH, C, 8, RPC, NG, 4, O), np.float32)
        for j in range(4):
            D[:, :, j:j + KW, :, :, j, :] = A[:, :, :, :, :, j, :]
        S = np.empty_like(D)  # slot rm holds dy=(rm-k)%5 -> roll dy by k
        for k in range(RPC):
            S[:, :, :, k] = np.roll(D[:, :, :, k], k, axis=0)
        wslab = np.zeros((128, RPC * NG * 4 * O), np.float32)
        wslab[:KH * NPL] = S.reshape(KH * NPL, -1)
        wslab[KH * NPL] = bc.reshape(-1)

        Uc = U[RPC * i:RPC * i + INR]  # [12, 24, FU]
        ufut = np.zeros((KH * NPL, 2 * FU), bf)
        for r in range(KH, INR):
            s, go = r % KH, (0 if r < 2 * KH else 1)
            ufut[s * NPL:(s + 1) * NPL, go * FU:(go + 1) * FU] = Uc[r]
        in_maps.append({
            "ui": np.ascontiguousarray(Uc[:KH].reshape(KH * NPL, FU)),
            "uf": ufut,
            "w": wslab.astype(bf),
        })
    return in_maps


def kernel(x, W, b):
    from concourse.bass_utils import run_bass_kernel_spmd

    nc = _get_nc()
    in_maps = _prep_inputs(x, W, b)
    br = run_bass_kernel_spmd(nc, in_maps, list(range(NCORES)),
                              **_cache.get("run_kwargs", {}))
    _cache["last_run"] = br
    oc = np.stack([np.asarray(br.results[i]["oc"]) for i in range(NCORES)])
    oc = oc.reshape(NCORES * RPC, 4, O, NG, B).astype(np.float32)
    out = oc.transpose(4, 2, 0, 3, 1).reshape(B, O, NCORES * RPC, RX)
    return np.ascontiguousarray(out[:, :, :RY, :])


# revision 8
# speedup vs baseline: 1.5956x; 1.1313x over previous
"""Locally-connected 2D layer on 8 Trainium2 NeuronCores.

Problem: x[128,3,64,64] f32, per-position weights W[60,60,32,75], bias b[60,60,32]
  out[b,o,y,x] = sum_k patches[b,y,x,k] * W[y,x,o,k] + b[y,x,o],  k=(c,dy,dx)

Strategy (spatial sharding over output rows, 8 rows/core, memory-regime):
  - Groups of 4 consecutive x positions share one full-array matmul: the
    contraction is the UNION of the 4 patch windows, planes (c, dy, dx') with
    dx' in 0..7 -> 5*3*8 = 120 partitions.  The stationary [120, 128] holds
    all 4 positions' weights ((j,o) columns, structural zeros where dx'-j is
    outside 0..4), so each [128, 128] matmul output is fully useful:
    out[(j,o), b] for 4 x-positions at once.  15 matmuls per row, 120 per
    core, N=128 streaming.  Bias is added on the host after gathering.
  - dy is handled with a mod-5 ring of row-slots (24 planes each); the per-row
    dy rotation is folded into the host-side W slab layout (np.roll), so the
    device always reads xp[0:120] as one fixed partition range.
  - Queue split matches measured rates (SWDGE ~280GB/s, both HWDGE queues
    combined ~120GB/s): gpsimd carries the initial ring fill, the W slab
    (four [128, 7680B] row-pair DMAs) and stores of rows 0-4; sync/scalar
    carry the staged future-row slabs, stores of rows 5-7, and the SBUF->SBUF
    ring advances (low latency, no HBM).
  - Output is bf16 (host upcasts); each row's [128, 3840B] store is split in
    two chunk-gated halves.  PSUM evacuation alternates vector/scalar.
  - ~24 dummy 1x512 matmuls on a scratch tile run during the initial DMA fill
    so the PE HAM clock-gate is warm (2.4GHz) when real matmuls start.
"""

import numpy as np

B, C, H, WIDTH = 128, 3, 64, 64
KH = KW = 5
RY = RX = 60
O = 32
K = 75
NCORES = 8
RPC = 8             # output rows computed per core (8*8=64, last 4 dropped)
INR = RPC + KH - 1  # 12 input rows per core
PADH = NCORES * RPC + KH - 1  # 68
NG = 15             # groups of 4 x-positions per row
NPL = C * 8         # 24 planes per ring slot (c, dx' in 0..7)
KP = KH * NPL       # 120 contraction partitions
FU = NG * B         # 1920 free elems per plane (g, b)
CHUNKS = ((0, 4), (4, 4), (8, 4), (12, 3))  # (first group, n groups) per PSUM chunk
NWARM = 24

_cache = {}


def _build():
    import concourse.bass as bass
    import concourse.bacc as bacc
    import concourse.tile as tile
    import concourse.mybir as mybir

    f32 = mybir.dt.float32
    din = mybir.dt.bfloat16
    nc = bacc.Bacc("TRN2", target_bir_lowering=False, debug=False,
                   num_devices=NCORES)
    ui_d = nc.dram_tensor("ui", [KP, FU], din, kind="ExternalInput")
    ufa_d = nc.dram_tensor("ufa", [KP, FU], din, kind="ExternalInput")
    ufb_d = nc.dram_tensor("ufb", [2 * NPL, FU], din, kind="ExternalInput")
    w_d = nc.dram_tensor("w", [128, RPC * NG * B], din, kind="ExternalInput")
    oc_d = nc.dram_tensor("oc", [RPC, 4, O, NG, B], din, kind="ExternalOutput")

    with tile.TileContext(nc) as tc:
        with (
            tc.tile_pool(name="const", bufs=1) as cpool,
            tc.tile_pool(name="os", bufs=3) as opool,
            tc.tile_pool(name="ps", bufs=4, space=bass.MemorySpace.PSUM) as ppool,
            tc.tile_pool(name="pw", bufs=1, space=bass.MemorySpace.PSUM) as wpool,
        ):
            xp = cpool.tile([KP, FU], din)            # ring planes
            ufa = cpool.tile([KP, FU], din)           # future rows 5-9
            ufb = cpool.tile([2 * NPL, FU], din)      # future rows 10-11
            ws = cpool.tile([128, RPC * NG * B], din)
            dm = cpool.tile([1, 512], din)            # warmup operand

            nc.gpsimd.dma_start(xp[:], ui_d[:])
            for p in range(4):  # W row-pairs on the gpsimd SWDGE queue
                c0, c1 = p * 2 * NG * B, (p + 1) * 2 * NG * B
                nc.gpsimd.dma_start(ws[:, c0:c1], w_d[:, c0:c1])
            nc.scalar.dma_start(ufa[:], ufa_d[:])
            nc.sync.dma_start(ufb[:], ufb_d[:])

            # PE warmup: keep the array busy during the fill so HAM
            # un-throttles the PE clock before the first real matmul
            nc.vector.memset(dm[:], 1.0)
            pw = wpool.tile([1, 512], f32)
            for _ in range(NWARM):
                nc.tensor.matmul(pw[0:1, :], dm[:, 0:1], dm[:, :])

            for k in range(RPC):
                ot = opool.tile([128, FU], din)
                oc_k = oc_d[k].rearrange("j o g b -> (j o) (g b)")
                st_eng = nc.gpsimd if k < 5 else (nc.sync, nc.scalar, nc.sync)[k - 5]
                for ci, (g0, gn) in enumerate(CHUNKS):
                    pt = ppool.tile([128, 4 * B], f32)
                    for gg in range(gn):
                        g = g0 + gg
                        nc.tensor.matmul(
                            pt[:, gg * B:(gg + 1) * B],
                            ws[0:KP, (k * NG + g) * B:(k * NG + g + 1) * B],
                            xp[:, g * B:(g + 1) * B],
                        )
                    if ci % 2 == 0:
                        nc.vector.tensor_copy(
                            ot[:, g0 * B:(g0 + gn) * B], pt[:, :gn * B])
                    else:
                        nc.scalar.copy(
                            ot[:, g0 * B:(g0 + gn) * B], pt[:, :gn * B])
                        f0, f1 = (0, 8 * B) if ci == 1 else (8 * B, FU)
                        if k + KH < INR:
                            # ring advance: slot k%5 <- input row k+5, copied
                            # SBUF->SBUF from the staged future slab
                            s = k % KH
                            src = (ufa[s * NPL:(s + 1) * NPL, f0:f1]
                                   if k + KH < 2 * KH else
                                   ufb[(k - KH) * NPL:(k - KH + 1) * NPL, f0:f1])
                            nc.sync.dma_start(
                                xp[s * NPL:(s + 1) * NPL, f0:f1], src)
                        st_eng.dma_start(oc_k[:, f0:f1], ot[:, f0:f1])

    nc.compile()
    return nc


def _get_nc():
    if "nc" not in _cache:
        _cache["nc"] = _build()
    return _cache["nc"]


def _prep_inputs(x, W, b):
    import ml_dtypes
    bf = ml_dtypes.bfloat16
    x = np.asarray(x, np.float32)
    W = np.asarray(W, np.float32)
    xh = np.zeros((PADH, C, WIDTH, B), np.float32)
    xh[:H] = x.transpose(2, 1, 3, 0)  # [row, c, w, batch]
    # union planes: U[row, (c,dx'), (g,b)] = xh[row, c, 4g+dx', b], dx' in 0..7
    U = np.zeros((PADH, C, 8, NG, B), np.float32)
    for dxp in range(8):
        U[:, :, dxp] = xh[:, :, dxp::4][:, :, :NG]
    U = U.reshape(PADH, NPL, FU).astype(bf)

    W5 = W.reshape(RY, RX, O, C, KH, KW)
    in_maps = []
    for i in range(NCORES):
        nk = min(RPC, RY - RPC * i)
        W5c = np.zeros((RPC, NG, 4, O, C, KH, KW), np.float32)
        W5c[:nk] = W5[RPC * i:RPC * i + nk].reshape(nk, NG, 4, O, C, KH, KW)
        A = W5c.transpose(5, 4, 6, 0, 1, 2, 3)  # [dy, c, dx, k, g, j, o]
        D = np.zeros((KH, C, 8, RPC, NG, 4, O), np.float32)
        for j in range(4):
            D[:, :, j:j + KW, :, :, j, :] = A[:, :, :, :, :, j, :]
        S = np.empty_like(D)  # slot rm holds dy=(rm-k)%5 -> roll dy by k
        for k in range(RPC):
            S[:, :, :, k] = np.roll(D[:, :, :, k], k, axis=0)
        wslab = np.zeros((128, RPC * NG * 4 * O), np.float32)
        wslab[:KP] = S.reshape(KP, -1)

        Uc = U[RPC * i:RPC * i + INR]  # [12, 24, FU]
        in_maps.append({
            "ui": np.ascontiguousarray(Uc[:KH].reshape(KP, FU)),
            "ufa": np.ascontiguousarray(Uc[KH:2 * KH].reshape(KP, FU)),
            "ufb": np.ascontiguousarray(Uc[2 * KH:].reshape(2 * NPL, FU)),
            "w": wslab.astype(bf),
        })
    return in_maps


def kernel(x, W, b):
    from concourse.bass_utils import run_bass_kernel_spmd

    nc = _get_nc()
    in_maps = _prep_inputs(x, W, b)
    br = run_bass_kernel_spmd(nc, in_maps, list(range(NCORES)),
                              **_cache.get("run_kwargs", {}))
    _cache["last_run"] = br
    oc = np.stack([np.asarray(br.results[i]["oc"]) for i in range(NCORES)])
    oc = oc.reshape(NCORES * RPC, 4, O, NG, B).astype(np.float32)
    out = oc.transpose(4, 2, 0, 3, 1).reshape(B, O, NCORES * RPC, RX)
    out = out[:, :, :RY, :] + np.asarray(b, np.float32).transpose(2, 0, 1)[None]
    return np.ascontiguousarray(out)
